# revision 1
# baseline (speedup 1.0000x reference)
"""DiffLogic network TRN2 kernel: 3 logic layers [B=256, W=64000] + GroupSum.

Sharding: pure data-parallel over batch across 8 cores (B=32/core), no
inter-core communication. Per core: activations h stored in DRAM as
[64000, 128] bf16 rows (256B, 32 real batch cols). Gathers a=h[idx_a],
b=h[idx_b] via SWDGE dma_gather with signed int16 indices (base at row
32000 so idx in [-32000, 32000)). Gate = c0+c1*a+c2*b+c3*ab computed on
DVE with stride-0 coefficient broadcasts; coefs = softmax(w)@G computed
on device (ACT exp + DVE reduce). GroupSum via PE one-hot matmul.
"""
import numpy as np
import ml_dtypes

import concourse.bass as bass
import concourse.tile as tile
import concourse.bacc as bacc
import concourse.mybir as mybir
from concourse.bass_utils import run_bass_kernel_spmd
from concourse.library_config import mlp
from concourse._compat import cdiv

W = 64000
BATCH = 256
NCORES = 8
BC = BATCH // NCORES        # 32 batch rows per core
IN_DIM = 1024
K = 10
TAU = 30.0
NSLOT = W // 128            # 500
E = 128                     # bf16 elements per h row (256B); [:32] real
CHUNK_SLOTS = 64            # neurons per chunk = 64*128 = 8192
GPN = 1024                  # idxs per dma_gather instruction
H_BASE = 32000              # gather base row (signed int16 rebase)

GATE_COEF = np.array([
    [0., 0., 0., 0.], [0., 0., 0., 1.], [0., 1., 0., -1.], [0., 1., 0., 0.],
    [0., 0., 1., -1.], [0., 0., 1., 0.], [0., 1., 1., -2.], [0., 1., 1., -1.],
    [1., -1., -1., 1.], [1., -1., -1., 2.], [1., 0., -1., 0.], [1., 0., -1., 1.],
    [1., -1., 0., 0.], [1., -1., 0., 1.], [1., 0., 0., -1.], [1., 0., 0., 0.],
], dtype=np.float32)  # [16, 4]

BF16 = mybir.dt.bfloat16
F32 = mybir.dt.float32
I16 = mybir.dt.int16
IDX_COLS = W // 16  # wrapped idx tensor cols per list

_NC_CACHE = {}


def _chunks():
    """Yield (slot0, nslots) chunks over the 500 slots."""
    s = 0
    while s < NSLOT:
        n = min(CHUNK_SLOTS, NSLOT - s)
        yield s, n
        s += n


def _gathers(nslots):
    """Split a chunk of nslots*128 idxs into per-instruction counts."""
    n = nslots * 128
    out = []
    while n > 0:
        g = min(GPN, n)
        out.append(g)
        n -= g
    return out


def build_nc():
    if "nc" in _NC_CACHE:
        return _NC_CACHE["nc"]
    nc = bacc.Bacc("TRN2", target_bir_lowering=False, debug=False,
                   enable_asserts=False, num_devices=NCORES)

    xT = nc.dram_tensor("xT", [IN_DIM, E], BF16, kind="ExternalInput")
    wf = [nc.dram_tensor(f"wf{l}", [128, NSLOT, 16], BF16, kind="ExternalInput")
          for l in range(3)]
    ia = [nc.dram_tensor(f"ia{l}", [128, IDX_COLS], I16, kind="ExternalInput")
          for l in range(3)]
    ib = [nc.dram_tensor(f"ib{l}", [128, IDX_COLS], I16, kind="ExternalInput")
          for l in range(3)]
    g10 = nc.dram_tensor("g10", [128, NSLOT, K], BF16, kind="ExternalInput")
    gmat = nc.dram_tensor("gmat", [128, 5, 16], BF16, kind="ExternalInput")
    h_dram = [nc.dram_tensor(f"h{l}", [W, E], BF16, kind="Internal")
              for l in range(2)]
    out_dram = nc.dram_tensor("out", [K, BC], F32, kind="ExternalOutput")

    with tile.TileContext(nc) as tc:
        with (
            tc.tile_pool(name="persist", bufs=1) as persist,
            tc.tile_pool(name="coef", bufs=1) as coefp,
            tc.tile_pool(name="gath", bufs=2) as gath,
            tc.tile_pool(name="temps", bufs=2) as temps,
            tc.tile_pool(name="psum", bufs=1, space="PSUM") as psump,
        ):
            nc.gpsimd.load_library(mlp)

            # persistent loads
            gmat_sb = persist.tile([128, 5, 16], BF16, tag="gmat")
            nc.sync.dma_start(gmat_sb[:], gmat[:])
            g10_sb = persist.tile([128, NSLOT, K], BF16, tag="g10")
            nc.sync.dma_start(g10_sb[:], g10[:])

            psum_out = psump.tile([K, BC], F32, tag="acc")
            n_mm = NSLOT  # total groupsum matmuls
            mm_i = 0

            for l in range(3):
                ia_sb = persist.tile([128, IDX_COLS], I16, tag="ia", name="ia_sb")
                ib_sb = persist.tile([128, IDX_COLS], I16, tag="ib", name="ib_sb")
                nc.sync.dma_start(ia_sb[:], ia[l][:])
                nc.sync.dma_start(ib_sb[:], ib[l][:])

                # ---- coefficient prep: coef = softmax(wf) @ GATE_COEF ----
                HS = NSLOT // 2
                cj = [coefp.tile([128, NSLOT], BF16, tag=f"c{j}", name=f"cj{j}") for j in range(4)]
                for h in range(2):
                    hs = slice(h * HS, (h + 1) * HS)
                    wf_sb = coefp.tile([128, HS, 16], BF16, tag="wf", name="wf_sb")
                    nc.sync.dma_start(wf_sb[:], wf[l][:, hs, :])
                    e_sb = coefp.tile([128, HS, 16], BF16, tag="e", name="e_sb")
                    nc.scalar.activation(e_sb[:], wf_sb[:],
                                         mybir.ActivationFunctionType.Exp)
                    prod = coefp.tile([128, HS, 16], BF16, tag="prod", name="prod")
                    craw = [coefp.tile([128, HS], F32, tag=f"craw{j}", name=f"craw{j}")
                            for j in range(4)]
                    for j in range(4):
                        gj = gmat_sb[:, j, :].unsqueeze(1).to_broadcast([128, HS, 16])
                        nc.vector.tensor_mul(prod[:], e_sb[:], gj)
                        nc.vector.tensor_reduce(craw[j][:], prod[:],
                                                mybir.AxisListType.X,
                                                mybir.AluOpType.add)
                    ssum = coefp.tile([128, HS], F32, tag="ssum", name="ssum")
                    nc.vector.tensor_reduce(ssum[:], e_sb[:], mybir.AxisListType.X,
                                            mybir.AluOpType.add)
                    rinv = coefp.tile([128, HS], F32, tag="rinv", name="rinv")
                    nc.vector.reciprocal(out=rinv[:], in_=ssum[:])
                    for j in range(4):
                        nc.vector.tensor_mul(cj[j][:, hs], craw[j][:], rinv[:])

                # ---- gather + gate over chunks ----
                if l == 0:
                    src_ap = xT[:]
                else:
                    src_ap = h_dram[l - 1][H_BASE:W]

                for s0, ns in _chunks():
                    a_t = gath.tile([128, CHUNK_SLOTS, E], BF16, tag="a")
                    b_t = gath.tile([128, CHUNK_SLOTS, E], BF16, tag="b")
                    col = s0 * 8  # idx cols consumed so far (128/16 per slot)
                    slot = 0
                    for n in _gathers(ns):
                        ncols = n // 16
                        nslots_g = n // 128
                        nc.gpsimd.dma_gather(
                            a_t[:, slot:slot + nslots_g, :], src_ap,
                            ia_sb[:, col:col + ncols], n, n, E)
                        nc.gpsimd.dma_gather(
                            b_t[:, slot:slot + nslots_g, :], src_ap,
                            ib_sb[:, col:col + ncols], n, n, E)
                        col += ncols
                        slot += nslots_g

                    av = a_t[:, :ns, :32]
                    bv = b_t[:, :ns, :32]

                    def cbc(j):
                        return (cj[j][:, s0:s0 + ns].unsqueeze(-1)
                                .to_broadcast([128, ns, 32]))

                    t_t = temps.tile([128, CHUNK_SLOTS, 32], BF16, tag="t")
                    u_t = temps.tile([128, CHUNK_SLOTS, 32], BF16, tag="u")
                    v_t = temps.tile([128, CHUNK_SLOTS, 32], BF16, tag="v")
                    w_t = temps.tile([128, CHUNK_SLOTS, 32], BF16, tag="w")
                    nc.vector.tensor_mul(t_t[:, :ns, :], av, bv)
                    nc.vector.tensor_mul(u_t[:, :ns, :], t_t[:, :ns, :], cbc(3))
                    nc.vector.tensor_mul(v_t[:, :ns, :], av, cbc(1))
                    nc.vector.tensor_mul(w_t[:, :ns, :], bv, cbc(2))
                    nc.vector.tensor_add(u_t[:, :ns, :], u_t[:, :ns, :], v_t[:, :ns, :])
                    nc.vector.tensor_add(w_t[:, :ns, :], w_t[:, :ns, :], cbc(0))
                    nc.vector.tensor_add(t_t[:, :ns, :], u_t[:, :ns, :], w_t[:, :ns, :])

                    if l < 2:
                        # write rows (s0+c)*128+p of h_dram[l]
                        hap = h_dram[l].ap()
                        dst = hap[s0 * 128: s0 * 128 + ns * 128, :32]
                        dst = dst.rearrange("(c p) e -> p c e", p=128)
                        nc.gpsimd.dma_start(dst, t_t[:, :ns, :])
                    else:
                        for c in range(ns):
                            nc.tensor.matmul(
                                psum_out[:],
                                lhsT=g10_sb[:, s0 + c, :],
                                rhs=t_t[:, c, :],
                                start=(mm_i == 0),
                                stop=(mm_i == n_mm - 1),
                            )
                            mm_i += 1

            out_sb = persist.tile([K, BC], F32, tag="outsb")
            nc.scalar.mul(out_sb[:], psum_out[:], 1.0 / TAU)
            nc.sync.dma_start(out_dram[:], out_sb[:])

    nc.compile()
    _NC_CACHE["nc"] = nc
    return nc


def _wrap(idx):
    """Flat idx list [n] -> [128, n/16] int16 wrapped per 16 partitions,
    replicated to the 8 gpsimd cores."""
    n = idx.shape[0]
    arr = np.empty((128, n // 16), dtype=np.int16)
    blk = idx.reshape(n // 16, 16).T.astype(np.int16)
    for g in range(8):
        arr[g * 16:(g + 1) * 16, :] = blk
    return arr


def _fix_trailing(idx_a, idx_b):
    """Ensure the last idx of every GPN-sublist is >= 0 for both lists
    (SWDGE trims trailing negatives). Returns permuted lists + perm."""
    perm = np.arange(W)
    a = idx_a.copy()
    b = idx_b.copy()
    pos = 0
    for s0, ns in _chunks():
        for n in _gathers(ns):
            last = pos + n - 1
            if a[last] < 0 or b[last] < 0:
                ok = np.nonzero((a[pos:last] >= 0) & (b[pos:last] >= 0))[0]
                j = pos + int(ok[-1])
                for arr in (a, b, perm):
                    arr[last], arr[j] = arr[j], arr[last]
            pos += n
    return a, b, perm


def _fold(x):
    """[W, ...] -> [128, NSLOT, ...] with row n=(c*128+p) at [p, c]."""
    return np.ascontiguousarray(
        x.reshape(NSLOT, 128, *x.shape[1:]).transpose(1, 0, *range(2, x.ndim + 1)))


def kernel(x, w1, w2, w3, idx_a1, idx_b1, idx_a2, idx_b2, idx_a3, idx_b3):
    x = np.asarray(x, dtype=np.float32)
    ws = [np.asarray(w, dtype=np.float32) for w in (w1, w2, w3)]
    ias = [np.asarray(i).astype(np.int64) for i in (idx_a1, idx_a2, idx_a3)]
    ibs = [np.asarray(i).astype(np.int64) for i in (idx_b1, idx_b2, idx_b3)]

    nc = build_nc()

    # ---- host-side index translation / layout prep (shared across cores) ----
    # layer 0: sources are x columns (0..1023), no rebase needed
    a0, b0, perm0 = ias[0].copy(), ibs[0].copy(), np.arange(W)
    perms = [perm0]
    lists = [(a0, b0)]
    for l in (1, 2):
        inv_prev = np.empty(W, dtype=np.int64)
        inv_prev[perms[l - 1]] = np.arange(W)
        ra = inv_prev[ias[l]] - H_BASE
        rb = inv_prev[ibs[l]] - H_BASE
        ra2, rb2, perm = _fix_trailing(ra, rb)
        perms.append(perm)
        lists.append((ra2, rb2))

    shared = {}
    for l in range(3):
        a, b = lists[l]
        shared[f"ia{l}"] = _wrap(a)
        shared[f"ib{l}"] = _wrap(b)
        shared[f"wf{l}"] = _fold(ws[l][perms[l]]).astype(ml_dtypes.bfloat16)

    group = perms[2] // (W // K)          # group id of neuron at list pos j
    g10 = np.zeros((W, K), dtype=np.float32)
    g10[np.arange(W), group] = 1.0
    shared["g10"] = _fold(g10).astype(ml_dtypes.bfloat16)

    gm = np.zeros((5, 16), dtype=np.float32)
    gm[:4] = GATE_COEF.T
    gm[4] = 1.0
    shared["gmat"] = np.broadcast_to(gm, (128, 5, 16)).astype(ml_dtypes.bfloat16)

    in_maps = []
    for c in range(NCORES):
        xc = x[c * BC:(c + 1) * BC]               # [32, 1024]
        xt = np.zeros((IN_DIM, E), dtype=ml_dtypes.bfloat16)
        xt[:, :BC] = xc.T.astype(ml_dtypes.bfloat16)
        m = dict(shared)
        m["xT"] = xt
        in_maps.append(m)

    res = run_bass_kernel_spmd(nc, in_maps, core_ids=list(range(NCORES)))

    out = np.empty((BATCH, K), dtype=np.float32)
    for c in range(NCORES):
        out[c * BC:(c + 1) * BC] = res.results[c]["out"].T
    return out



# revision 13
# speedup vs baseline: 1.2239x; 1.2239x over previous
"""DiffLogic network TRN2 kernel: 3 logic layers [B=256, W=64000] + GroupSum.

Sharding: pure data-parallel over batch across 8 cores (B=32/core), no
inter-core communication. Per core: activations h stored in DRAM as
[64000, 128] bf16 rows (256B stride — SWDGE requires 256B-aligned row
strides) with only [:32] (64B) used. Gathers a=h[idx_a], b=h[idx_b] via
SWDGE dma_gather with the instruction's elem_size narrowed to 32 elements
(64B) post-build: the ucode moves 64B per descriptor into a packed output
tile, and 64B descriptors hit the DMA min-transfer floor (7ns vs 22.75ns
for 256B) — 3.25x cheaper than the 256B-elem gather.

Gate algebra host-folded: coef = softmax(w) @ GATE_COEF on host; each
layer's constant c0 folds into the next layer's coefficients (device
computes s = c1'*a + c2'*b + c3'*a*b only — 5 DVE ops). Layer 3 fuses with
GroupSum on the PE: out_k = sum_s W1*a + W2*b + W3*(ab) via per-column
matmuls into three PSUM accumulators; dropped constants re-added on host.
"""
import numpy as np
import ml_dtypes

import concourse.bass as bass
import concourse.tile as tile
import concourse.bacc as bacc
import concourse.mybir as mybir
from concourse.bass_utils import run_bass_kernel_spmd
from concourse.library_config import mlp

W = 64000
BATCH = 256
NCORES = 8
BC = BATCH // NCORES        # 32 batch rows per core
IN_DIM = 1024
K = 10
TAU = 30.0
NSLOT = W // 128            # 500 slot-columns
E = 128                     # bf16 elements per table row stride (256B)
EV = 32                     # valid elements per row (64B payload)
CHUNK_SLOTS = 64            # slot-columns per chunk (8192 idxs per gather)
GPN = 1024                  # idxs per dma_gather instruction
H_BASE = 32000              # gather base row (signed int16 rebase)

GATE_COEF = np.array([
    [0., 0., 0., 0.], [0., 0., 0., 1.], [0., 1., 0., -1.], [0., 1., 0., 0.],
    [0., 0., 1., -1.], [0., 0., 1., 0.], [0., 1., 1., -2.], [0., 1., 1., -1.],
    [1., -1., -1., 1.], [1., -1., -1., 2.], [1., 0., -1., 0.], [1., 0., -1., 1.],
    [1., -1., 0., 0.], [1., -1., 0., 1.], [1., 0., 0., -1.], [1., 0., 0., 0.],
], dtype=np.float64)  # [16, 4] = (c0, c1, c2, c3) per gate id

BF16 = mybir.dt.bfloat16
F32 = mybir.dt.float32
I16 = mybir.dt.int16
IDX_COLS = W // 16          # wrapped idx tensor cols per list

_NC_CACHE = {}


def _chunks():
    s = 0
    while s < NSLOT:
        n = min(CHUNK_SLOTS, NSLOT - s)
        yield s, n
        s += n


def _gathers(nslots):
    n = nslots * 128
    out = []
    while n > 0:
        g = min(GPN, n)
        out.append(g)
        n -= g
    return out


def _gather64(nc, flat_tile, src_ap, idx_ap, n, off):
    """SWDGE gather of n rows, 64B (EV=32 bf16 elems) each, packed output.

    `flat_tile` is a [128, CHUNK_SLOTS*E] tile. The instruction is built
    claiming elem_size=E over a [128, n/128, E] view (passes the
    256B-multiple check), then elem_size is narrowed to EV: the ucode
    moves EV elems per idx and packs the destination, so the real data
    lands as [128, n/128, EV] at free-byte offset off*EV*2 of the tile
    (off = idxs already gathered into this tile)."""
    ns = n // 128
    base = off // 128 * EV
    # claim exactly the packed byte range: build with num_idxs=n/4 so the
    # [128, n/512, E] view passes the shape checks, then restore num_idxs.
    view = (flat_tile[:, base:base + ns * EV]
            .rearrange("p (a e) -> p a e", e=E))
    inst = nc.gpsimd.dma_gather(view, src_ap, idx_ap, n // 4, n, E)
    inst.ins.elem_size = EV
    inst.ins.num_idxs = n
    return inst


def build_nc():
    if "nc" in _NC_CACHE:
        return _NC_CACHE["nc"]
    nc = bacc.Bacc("TRN2", target_bir_lowering=False, debug=False,
                   enable_asserts=False, num_devices=NCORES)

    xT = nc.dram_tensor("xT", [IN_DIM, E], BF16, kind="ExternalInput")
    ia = [nc.dram_tensor(f"ia{l}", [128, IDX_COLS], I16, kind="ExternalInput")
          for l in range(3)]
    ib = [nc.dram_tensor(f"ib{l}", [128, IDX_COLS], I16, kind="ExternalInput")
          for l in range(3)]
    cf = [nc.dram_tensor(f"cf{l}", [128, NSLOT, 3], BF16, kind="ExternalInput")
          for l in range(2)]
    wm = [nc.dram_tensor(f"wm{t}", [128, NSLOT, K], BF16, kind="ExternalInput")
          for t in range(3)]
    h_dram = [nc.dram_tensor(f"h{l}", [W, E], BF16, kind="Internal")
              for l in range(2)]
    out_dram = nc.dram_tensor("out", [K, BC], F32, kind="ExternalOutput")

    with tile.TileContext(nc) as tc:
        with (
            tc.tile_pool(name="persist", bufs=1) as persist,
            tc.tile_pool(name="gath", bufs=2) as gath,
            tc.tile_pool(name="temps", bufs=2) as temps,
            tc.tile_pool(name="psum", bufs=1, space="PSUM") as psump,
        ):
            nc.gpsimd.load_library(mlp)

            cf_sb = []
            for l in range(2):
                t_c = persist.tile([128, NSLOT, 3], BF16, tag=f"cf{l}",
                                   name=f"cf{l}")
                nc.sync.dma_start(t_c[:], cf[l][:])
                cf_sb.append(t_c)
            wm_sb = []
            for t in range(3):
                t_w = persist.tile([128, NSLOT, K], BF16, tag=f"wm{t}",
                                   name=f"wm{t}")
                nc.sync.dma_start(t_w[:], wm[t][:])
                wm_sb.append(t_w)

            ps = [psump.tile([K, BC], F32, tag=f"ps{t}", name=f"ps{t}")
                  for t in range(3)]

            for l in range(3):
                ia_sb = persist.tile([128, IDX_COLS], I16, tag="ia", name="ia_sb")
                ib_sb = persist.tile([128, IDX_COLS], I16, tag="ib", name="ib_sb")
                nc.sync.dma_start(ia_sb[:], ia[l][:])
                nc.sync.dma_start(ib_sb[:], ib[l][:])

                src_ap = xT[:] if l == 0 else h_dram[l - 1][H_BASE:W]

                for s0, ns in _chunks():
                    a_t = gath.tile([128, CHUNK_SLOTS * E], BF16, tag="a")
                    b_t = gath.tile([128, CHUNK_SLOTS * E], BF16, tag="b")
                    col = s0 * 8
                    off = 0
                    for n in _gathers(ns):
                        _gather64(nc, a_t, src_ap,
                                  ia_sb[:, col:col + n // 16], n, off)
                        _gather64(nc, b_t, src_ap,
                                  ib_sb[:, col:col + n // 16], n, off)
                        col += n // 16
                        off += n
                    av = (a_t[:, :ns * 128 * EV // 128]
                          .rearrange("p (a e) -> p a e", e=EV))
                    bv = (b_t[:, :ns * 128 * EV // 128]
                          .rearrange("p (a e) -> p a e", e=EV))
                    if l < 2:
                        def bc(j):
                            return (cf_sb[l][:, s0:s0 + ns, j].unsqueeze(-1)
                                    .to_broadcast([128, ns, EV]))
                        u_t = temps.tile([128, CHUNK_SLOTS, EV], BF16, tag="u")
                        v_t = temps.tile([128, CHUNK_SLOTS, EV], BF16, tag="v")
                        o_t = temps.tile([128, CHUNK_SLOTS, EV], BF16, tag="o")
                        # u = (c3*b + c1) * a ; v = c2*b ; o = u + v
                        nc.vector.tensor_mul(u_t[:, :ns, :], bv, bc(2))
                        nc.vector.tensor_add(u_t[:, :ns, :], u_t[:, :ns, :], bc(0))
                        nc.vector.tensor_mul(u_t[:, :ns, :], u_t[:, :ns, :], av)
                        nc.vector.tensor_mul(v_t[:, :ns, :], bv, bc(1))
                        nc.vector.tensor_add(o_t[:, :ns, :], u_t[:, :ns, :],
                                             v_t[:, :ns, :])
                        hap = h_dram[l].ap()
                        dst = hap[s0 * 128: s0 * 128 + ns * 128, :EV]
                        dst = dst.rearrange("(c p) e -> p c e", p=128)
                        nc.sync.dma_start(dst, o_t[:, :ns, :])
                    else:
                        t_t = temps.tile([128, CHUNK_SLOTS, EV], BF16, tag="t")
                        nc.vector.tensor_mul(t_t[:, :ns, :], av, bv)
                        rhs = (av, bv, t_t[:, :ns, :])
                        for c in range(ns):
                            gc = s0 + c
                            for t in range(3):
                                nc.tensor.matmul(
                                    ps[t][:],
                                    lhsT=wm_sb[t][:, gc, :],
                                    rhs=rhs[t][:, c, :],
                                    start=(gc == 0),
                                    stop=(gc == NSLOT - 1),
                                )

            s_t = persist.tile([K, BC], F32, tag="s")
            nc.vector.tensor_copy(s_t[:], ps[0][:])
            nc.vector.tensor_add(s_t[:], s_t[:], ps[1][:])
            nc.vector.tensor_add(s_t[:], s_t[:], ps[2][:])
            nc.sync.dma_start(out_dram[:], s_t[:])

    nc.compile()
    _NC_CACHE["nc"] = nc
    return nc


def _wrap(idx):
    """Flat idx list [n] -> [128, n/16] int16 wrapped per 16 partitions,
    replicated to the 8 gpsimd cores."""
    n = idx.shape[0]
    arr = np.empty((128, n // 16), dtype=np.int16)
    blk = idx.reshape(n // 16, 16).T.astype(np.int16)
    for g in range(8):
        arr[g * 16:(g + 1) * 16, :] = blk
    return arr


def _fix_trailing(idx_a, idx_b):
    """Ensure the last idx of every GPN-sublist is >= 0 for both lists
    (SWDGE trims trailing negatives). Returns permuted lists + perm."""
    perm = np.arange(W)
    a = idx_a.copy()
    b = idx_b.copy()
    pos = 0
    for s0, ns in _chunks():
        for n in _gathers(ns):
            last = pos + n - 1
            if a[last] < 0 or b[last] < 0:
                ok = np.nonzero((a[pos:last] >= 0) & (b[pos:last] >= 0))[0]
                j = pos + int(ok[-1])
                for arr in (a, b, perm):
                    arr[last], arr[j] = arr[j], arr[last]
            pos += n
    return a, b, perm


def _fold(x):
    """[W, ...] -> [128, NSLOT, ...] with neuron-at-listpos n=(c*128+p) at
    [p, c]."""
    return np.ascontiguousarray(
        x.reshape(NSLOT, 128, *x.shape[1:]).transpose(1, 0, *range(2, x.ndim + 1)))


def kernel(x, w1, w2, w3, idx_a1, idx_b1, idx_a2, idx_b2, idx_a3, idx_b3):
    x = np.asarray(x, dtype=np.float32)
    ws = [np.asarray(w, dtype=np.float64) for w in (w1, w2, w3)]
    ias = [np.asarray(i).astype(np.int64) for i in (idx_a1, idx_a2, idx_a3)]
    ibs = [np.asarray(i).astype(np.int64) for i in (idx_b1, idx_b2, idx_b3)]

    nc = build_nc()

    # host: coef = softmax(w) @ GATE_COEF in float64, then fold each layer's
    # constant term into the next layer's coefficients.
    cofs = []
    for wl in ws:
        e = np.exp(wl - wl.max(axis=1, keepdims=True))
        sm = e / e.sum(axis=1, keepdims=True)
        cofs.append(sm @ GATE_COEF)  # [W, 4]

    # list-position permutations per layer (from trailing-negative fixes):
    # layer l's neuron at list position j is original neuron perms[l][j].
    # h row of list position j is (j%...): row = (chunk-major) j -> handled
    # by the baseline numbering row(listpos j) = j with table order j.
    shared = {}
    perms = []
    lists = []
    a0, b0 = ias[0].copy(), ibs[0].copy()
    perms.append(np.arange(W))
    lists.append((a0, b0))
    for l in (1, 2):
        inv_prev = np.empty(W, dtype=np.int64)
        inv_prev[perms[l - 1]] = np.arange(W)
        ra = inv_prev[ias[l]] - H_BASE
        rb = inv_prev[ibs[l]] - H_BASE
        ra2, rb2, perm = _fix_trailing(ra, rb)
        perms.append(perm)
        lists.append((ra2, rb2))

    d_prev = None
    dsum = None
    for l in range(3):
        p = perms[l]
        c0, c1, c2, c3 = cofs[l][p].T         # coefs in list order
        ia_orig = ias[l][p]                    # original prev-layer ids
        ib_orig = ibs[l][p]
        if l == 0:
            ga = gb = np.zeros(W)
        else:
            ga, gb = d_prev_orig[ia_orig], d_prev_orig[ib_orig]
        c1p = c1 + c3 * gb
        c2p = c2 + c3 * ga
        d_list = c0 + c1 * ga + c2 * gb + c3 * ga * gb   # in list order
        # map back to original neuron ids for the next layer's lookups
        d_prev_orig = np.empty(W)
        d_prev_orig[p] = d_list
        a, b = lists[l]
        shared[f"ia{l}"] = _wrap(a)
        shared[f"ib{l}"] = _wrap(b)
        if l < 2:
            cf_v = np.stack([c1p, c2p, c3], axis=1)
            shared[f"cf{l}"] = _fold(cf_v.astype(ml_dtypes.bfloat16))
        else:
            group = p // (W // K)              # group of neuron at list pos
            onehot = np.zeros((W, K))
            onehot[np.arange(W), group] = 1.0
            for t, cv in enumerate((c1p, c2p, c3)):
                shared[f"wm{t}"] = _fold(
                    (onehot * cv[:, None]).astype(ml_dtypes.bfloat16))
            dsum = np.zeros(K)
            np.add.at(dsum, group, d_list)

    in_maps = []
    for c in range(NCORES):
        xc = x[c * BC:(c + 1) * BC]               # [32, 1024]
        xt = np.zeros((IN_DIM, E), dtype=ml_dtypes.bfloat16)
        xt[:, :BC] = xc.T.astype(ml_dtypes.bfloat16)
        m = dict(shared)
        m["xT"] = xt
        in_maps.append(m)

    res = run_bass_kernel_spmd(nc, in_maps, core_ids=list(range(NCORES)))

    out = np.empty((BATCH, K), dtype=np.float32)
    for c in range(NCORES):
        psv = res.results[c]["out"].astype(np.float64)  # [K, BC]
        out[c * BC:(c + 1) * BC] = ((psv + dsum[:, None]) / TAU).T
    return out
